# revision 14
# baseline (speedup 1.0000x reference)
"""CVQNN classifier kernel for 8 Trainium2 NeuronCores.

Math: the whole quantum circuit collapses to a batch-independent affine map
(S, d) on 128-dim phase space.  Per batch row the heavy work is
    m = x @ W2 + d20              (W2 = S[rows, :64].T, shape (64, 20))
    out_k = log1p(m_x[k]^2 + m_p[k]^2 + cov_k/4 - 0.5)
i.e. a (B,64) @ (64,20) matmul + elementwise tail -> (B,10).  Memory bound:
minimize HBM bytes (fp16 in, fp16 out; gate is 2e-2, fp16 end-to-end is
~5e-4) and keep the 16 DMA engines saturated end-to-end.

Key algebra: (p+d)^2 = p^2 + 2 d.p + d^2, so the affine offset d never
needs a separate elementwise add.  The linear cross term 2(dx px + dp pp)
is a linear functional of x and rides the matmul as 10 extra columns per
block; d^2 folds into the precomputed constant.  The tail is then
    sq = p^2           (ACT Square, reads PSUM directly)
    s  = sq_x + sq_p   (DVE, fp16 2x)
    u  = s + cross     (DVE, reads PSUM cross columns)
    v  = u + c'        (DVE, fp16 2x;  c' = cov/4 - 1/2 + dx^2 + dp^2)
    o  = ln(1+v)       (ACT, fp16 out)
relu is dropped: the true nmean >= 0 exactly (mean photon number), so v
only ever dips below 0 by rounding ~1e-3, where ln(1+v) ~ v is within
tolerance of the reference's 0.

Device layout (per core, R = 125184 rows = 489 pair-blocks of 256):
  - host packs xstack (128, R/2) fp16, "2-pack": column c = (pair b,
    lane l), partitions 0..63 = features of row 256b+l, partitions
    64..127 = features of row 256b+128+l.  Full 128 partitions keeps
    DMA descriptors balanced on all 16 engines (a 65-partition layout
    was measured to use only 13) and halves the LDWEIGHTS count.
  - per super-block of `jblk` pair-blocks (32 = 4 psum banks, double
    buffered): 1 input DMA [128, 128*jblk] fp16, one matmul per
    pair-block: stationary = xstack_b [128, 128], moving = w60
    [128, 60] = [[W2,0],[0,W2],[CW,0],[0,CW]], psum block =
    [Ax Ap Bx Bp | crossA crossB] (8 blocks of 60 per 512-col bank).
  - out DMA [128, 20*jblk] fp16 issues on the gpsimd SWDGE queue so
    neither the input (sync) nor the ACT queue ever waits on it.
  - the v/ln/out tail of super-block n-1 is emitted during super-block
    n (software pipelining) so no op ever stalls at the head of its
    in-order engine queue waiting on a cross-engine dependency.
  - widths taper [16, 32*13, 24, 16, 9, 8]: small first block starts
    compute early; small last blocks shrink the post-stream drain.
"""

import numpy as np

import concourse.bacc as bacc
import concourse.mybir as mybir
import concourse.tile as tile
from concourse.bass_utils import run_bass_kernel_spmd

N = 64          # wires
OUT = 10        # measured wires / classes
NCORES = 8
PPB = 8                        # pair-blocks per psum bank (8*60 = 480 cols)
SBP = 4 * PPB                  # pair-blocks per full super-block (32)
WIDTHS = [16] + [32] * 13 + [24, 16, 9, 8]    # pair-blocks per super-block
NP2 = sum(WIDTHS)              # 489 pair-blocks
NJ = 2 * NP2                   # 978 j-blocks of 128 rows
R = 128 * NJ                   # per-core rows = 125184
B_PAD = R * NCORES             # 1001472
F32 = mybir.dt.float32
F16 = mybir.dt.float16
NPF16 = np.float16


# ---------------------------------------------------------------- host math
def _bs_pass(n, start, int_params):
    i = np.arange(start, n - 1, 2)
    j = i + 1
    theta = int_params[3 * i]
    phi = int_params[3 * i + 1]
    ct, st = np.cos(theta), np.sin(theta)
    cp, sp = np.cos(phi), np.sin(phi)
    S = np.eye(2 * n)
    S[i, i] = ct
    S[i, j] = -cp * st
    S[i, n + j] = -sp * st
    S[j, i] = cp * st
    S[j, j] = ct
    S[j, n + i] = -sp * st
    S[n + i, j] = sp * st
    S[n + i, n + i] = ct
    S[n + i, n + j] = -cp * st
    S[n + j, i] = sp * st
    S[n + j, n + i] = cp * st
    S[n + j, n + j] = ct
    return S


def _layer_symplectic(n, int1, squeezes, int2):
    M = _bs_pass(n, 0, int1)
    M = _bs_pass(n, 1, int1) @ M
    c = np.concatenate([np.cos(int1[2::3]), np.ones(1)])
    s = np.concatenate([np.sin(int1[2::3]), np.zeros(1)])
    Rm = np.block([[np.diag(c), np.diag(-s)], [np.diag(s), np.diag(c)]])
    Sq = np.diag(np.concatenate([np.exp(-squeezes), np.exp(squeezes)]))
    M = Sq @ (Rm @ M)
    M = _bs_pass(n, 0, int2) @ M
    M = _bs_pass(n, 1, int2) @ M
    return M


def _affine_map(layers):
    n = N
    S = np.eye(2 * n)
    d = np.zeros(2 * n)
    for int1, sq, int2, disp in layers:
        M = _layer_symplectic(n, int1, sq, int2)
        S = M @ S
        d = M @ d
        d[:n] += 2.0 * disp
    return S, d


def _device_constants(layers):
    S, d = _affine_map(layers)
    w = np.arange(OUT)
    rows = np.concatenate([w, N + w])
    cov = S @ S.T
    cov_term = cov[w, w] + cov[N + w, N + w]            # (10,)
    W2 = S[rows, :N].T                                  # (64, 20)
    d20 = d[rows] / 2.0                                 # (20,)
    # cross weights: CW[i,k] = 2(dx_k W2[i,k] + dp_k W2[i,10+k])
    CW = 2.0 * (W2[:, 0:OUT] * d20[0:OUT] + W2[:, OUT:] * d20[OUT:])
    # c' = cov/4 - 1/2 + dx^2 + dp^2
    cmod = cov_term / 4.0 - 0.5 + d20[0:OUT] ** 2 + d20[OUT:] ** 2

    w60 = np.zeros((128, 60), NPF16)
    w60[0:64, 0:20] = W2.astype(NPF16)
    w60[64:128, 20:40] = W2.astype(NPF16)
    w60[0:64, 40:50] = CW.astype(NPF16)
    w60[64:128, 50:60] = CW.astype(NPF16)

    cconst = np.ascontiguousarray(np.broadcast_to(
        np.tile(cmod.astype(NPF16), 2 * SBP), (128, 20 * SBP))).astype(NPF16)
    return w60, cconst


# ---------------------------------------------------------------- bass build
def build_nc(widths=None):
    widths = widths or WIDTHS
    np2 = sum(widths)
    cc = 128 * np2                              # xstack cols
    nc = bacc.Bacc("TRN2", target_bir_lowering=False)
    OC = 20 * SBP                               # out cols per full SB (640)
    xs = nc.dram_tensor("xs", (128, cc), F16, kind="ExternalInput")
    wst = nc.dram_tensor("w60", (128, 60), F16, kind="ExternalInput")
    ccon = nc.dram_tensor("covconst", (128, OC), F16, kind="ExternalInput")
    out = nc.dram_tensor("out", (128, 2 * np2 * OUT), F16,
                         kind="ExternalOutput")

    Square = mybir.ActivationFunctionType.Square
    Ln = mybir.ActivationFunctionType.Ln

    with tile.TileContext(nc) as tc:
        with (
            tc.tile_pool(name="const", bufs=1) as cpool,
            tc.tile_pool(name="xin", bufs=8) as xpool,
            tc.tile_pool(name="mid", bufs=4) as mpool,
            tc.tile_pool(name="ob", bufs=4) as opool,
            tc.tile_pool(name="ps", bufs=2, space="PSUM") as pspool,
        ):
            # consts on the scalar queue: the sync queue is a pure x stream
            w_t = cpool.tile([128, 60], F16)
            nc.scalar.dma_start(w_t[:], wst[:])
            c_t = cpool.tile([128, OC], F16)
            nc.scalar.dma_start(c_t[:], ccon[:])

            def emit_tail(prev):
                # tail of the PREVIOUS super-block: its u is long done, so
                # none of these ops stalls at the head of an in-order
                # engine queue (a stalled op blocks everything emitted
                # after it on the same queue).
                u, col_base, jblk = prev
                oc = 20 * jblk
                v = mpool.tile([128, oc], F16, tag="v")
                nc.vector.tensor_add(v[:], u[:], c_t[:, 0:oc])
                o = opool.tile([128, oc], F16, tag="o")
                nc.scalar.activation(o[:], v[:], Ln, bias=1.0)
                ob = (col_base // 128) * 20
                nc.gpsimd.dma_start(out[:, ob:ob + oc], o[:])

            def emit_sb(col_base, jblk, in_chunks, prev):
                w = 128 * jblk
                tin = xpool.tile([128, w], F16, tag="tin")
                q = w // in_chunks
                for c4 in range(in_chunks):
                    nc.sync.dma_start(
                        tin[:, c4 * q:(c4 + 1) * q],
                        xs[:, col_base + c4 * q:col_base + (c4 + 1) * q])

                if prev is not None:
                    emit_tail(prev)

                # psum: 8 pair-blocks of 60 cols in the first 480 cols of
                # each 512-col bank (no bank crossing)
                nbank = (jblk + PPB - 1) // PPB
                ps = pspool.tile([128, nbank, 512], F32, tag="ps")
                for j in range(jblk):
                    nc.tensor.matmul(
                        ps[:, j // PPB, 60 * (j % PPB):60 * (j % PPB) + 60],
                        tin[:, 128 * j:128 * j + 128], w_t[:],
                        start=True, stop=True,
                    )
                fullb = jblk // PPB          # banks fully used
                remg = jblk - fullb * PPB    # pair-blocks in the ragged bank
                sq = mpool.tile([128, 40 * jblk], F16, tag="sq")
                if fullb:
                    pv = ps[:, 0:fullb, 0:60 * PPB].rearrange(
                        "p t (g c) -> p t g c", c=60)[:, :, :, 0:40]
                    sv = sq[:, 0:40 * PPB * fullb].rearrange(
                        "p (t g c) -> p t g c", g=PPB, c=40)
                    nc.scalar.activation(sv, pv, Square)
                if remg:
                    pv = ps[:, fullb, 0:60 * remg].rearrange(
                        "p (g c) -> p g c", c=60)[:, :, 0:40]
                    sv = sq[:, 40 * PPB * fullb:40 * jblk].rearrange(
                        "p (g c) -> p g c", c=40)
                    nc.scalar.activation(sv, pv, Square)

                oc = 20 * jblk
                sqv = sq[:].rearrange("p (g r k) -> p g r k", r=2, k=OUT)
                s = mpool.tile([128, oc], F16, tag="s")
                sv = s[:].rearrange("p (g k) -> p g k", k=OUT)
                nc.vector.tensor_add(sv, sqv[:, :, 0, :], sqv[:, :, 1, :])
                u = mpool.tile([128, oc], F16, tag="u")
                if fullb:
                    pc = ps[:, 0:fullb, 0:60 * PPB].rearrange(
                        "p t (g c) -> p t g c", c=60)[:, :, :, 40:60]
                    uv = u[:, 0:20 * PPB * fullb].rearrange(
                        "p (t g c) -> p t g c", g=PPB, c=20)
                    sv2 = s[:, 0:20 * PPB * fullb].rearrange(
                        "p (t g c) -> p t g c", g=PPB, c=20)
                    nc.vector.tensor_add(uv, sv2, pc)
                if remg:
                    pc = ps[:, fullb, 0:60 * remg].rearrange(
                        "p (g c) -> p g c", c=60)[:, :, 40:60]
                    uv = u[:, 20 * PPB * fullb:oc].rearrange(
                        "p (g c) -> p g c", c=20)
                    sv2 = s[:, 20 * PPB * fullb:oc].rearrange(
                        "p (g c) -> p g c", c=20)
                    nc.vector.tensor_add(uv, sv2, pc)
                return (u, col_base, jblk)

            # first tile's DMA in halves so compute starts sooner
            col = 0
            prev = None
            for i, wdt in enumerate(widths):
                prev = emit_sb(col, wdt, 2 if i == 0 else 1, prev)
                col += 128 * wdt
            emit_tail(prev)
    nc.compile()
    return nc


# ---------------------------------------------------------------- host glue
def _make_in_maps(x_batch, w60, cconst):
    B = x_batch.shape[0]
    xpad = np.zeros((B_PAD, N), NPF16)
    xpad[:B] = x_batch
    in_maps = []
    for c in range(NCORES):
        xc = xpad[c * R:(c + 1) * R]
        # xstk[64*m + f, 128*b + l] = xc[256*b + 128*m + l, f]
        xstk = np.ascontiguousarray(
            xc.reshape(R // 256, 2, 128, N).transpose(1, 3, 0, 2)
            .reshape(128, R // 2))
        in_maps.append({"xs": xstk, "w60": w60, "covconst": cconst})
    return in_maps


def _decode_out(results, B):
    full = np.empty((B_PAD, OUT), np.float32)
    for c in range(NCORES):
        O = results[c]["out"].astype(np.float32).reshape(128, NJ, OUT)
        rows = O.transpose(1, 0, 2).reshape(R, OUT)
        full[c * R:(c + 1) * R] = rows
    return full[:B]


_NC_CACHE = {}


def kernel(x_batch, int1_0, squeezes_0, int2_0, disp_0,
           int1_1, squeezes_1, int2_1, disp_1, _trace=False):
    layers = [
        (np.asarray(int1_0, np.float64), np.asarray(squeezes_0, np.float64),
         np.asarray(int2_0, np.float64), np.asarray(disp_0, np.float64)),
        (np.asarray(int1_1, np.float64), np.asarray(squeezes_1, np.float64),
         np.asarray(int2_1, np.float64), np.asarray(disp_1, np.float64)),
    ]
    w60, cconst = _device_constants(layers)
    in_maps = _make_in_maps(np.asarray(x_batch, np.float32), w60, cconst)

    if "nc" not in _NC_CACHE:
        _NC_CACHE["nc"] = build_nc()
    nc = _NC_CACHE["nc"]

    res = run_bass_kernel_spmd(
        nc, in_maps, core_ids=list(range(NCORES)), trace=_trace
    )
    out = _decode_out(res.results, x_batch.shape[0])
    if _trace:
        return out, res
    return out


# revision 17
# speedup vs baseline: 1.0854x; 1.0854x over previous
"""CVQNN classifier kernel for 8 Trainium2 NeuronCores.

Math: the whole quantum circuit collapses to a batch-independent affine map
(S, d) on 128-dim phase space.  Per batch row the heavy work is
    m = x @ W2 + d20              (W2 = S[rows, :64].T, shape (64, 20))
    out_k = log1p(m_x[k]^2 + m_p[k]^2 + cov_k/4 - 0.5)
i.e. a (B,64) @ (64,20) matmul + elementwise tail -> (B,10).  Memory bound:
minimize HBM bytes (fp16 in, fp16 out; gate is 2e-2, fp16 end-to-end is
~5e-4) and keep the 16 DMA engines saturated end-to-end.

Key algebra: (p+d)^2 = p^2 + 2 d.p + d^2, so the affine offset d never
needs a separate elementwise add.  The linear cross term 2(dx px + dp pp)
is a linear functional of x and rides the matmul as 10 extra columns per
block; d^2 folds into the precomputed constant.  The tail is then
    sq = p^2           (ACT Square, reads PSUM directly)
    s  = sq_x + sq_p   (DVE, fp16 2x)
    u  = s + cross     (DVE, reads PSUM cross columns)
    v  = u + c'        (DVE, fp16 2x;  c' = cov/4 - 1/2 + dx^2 + dp^2)
    o  = ln(1+v)       (ACT, fp16 out)
relu is dropped: the true nmean >= 0 exactly (mean photon number), so v
only ever dips below 0 by rounding ~1e-3, where ln(1+v) ~ v is within
tolerance of the reference's 0.

Device layout (per core, R = 125184 rows = 489 pair-blocks of 256):
  - host packs xstack (128, R/2) fp16, "2-pack": column c = (pair b,
    lane l), partitions 0..63 = features of row 256b+l, partitions
    64..127 = features of row 256b+128+l.  Full 128 partitions keeps
    DMA descriptors balanced on all 16 engines (a 65-partition layout
    was measured to use only 13) and halves the LDWEIGHTS count.
  - per super-block of `jblk` pair-blocks (32 = 4 psum banks, double
    buffered): 1 input DMA [128, 128*jblk] fp16, one matmul per
    pair-block: stationary = xstack_b [128, 128], moving = w60
    [128, 60] = [[W2,0],[0,W2],[CW,0],[0,CW]], psum block =
    [Ax Ap Bx Bp | crossA crossB] (8 blocks of 60 per 512-col bank).
  - out DMA [128, 20*jblk] fp16 issues on the gpsimd SWDGE queue so
    neither the input (sync) nor the ACT queue ever waits on it.
  - the v/ln/out tail of super-block n-1 is emitted during super-block
    n (software pipelining) so no op ever stalls at the head of its
    in-order engine queue waiting on a cross-engine dependency.
  - widths taper [16, 32*13, 24, 16, 9, 8]: small first block starts
    compute early; small last blocks shrink the post-stream drain.
"""

import numpy as np

import concourse.bacc as bacc
import concourse.mybir as mybir
import concourse.tile as tile
from concourse.bass_utils import run_bass_kernel_spmd

N = 64          # wires
OUT = 10        # measured wires / classes
NCORES = 8
PPB = 8                        # pair-blocks per psum bank (8*60 = 480 cols)
CHK = 2 * PPB                  # pair-blocks per psum chunk (2 banks)
SBP = 48                       # pair-blocks per full super-block
WIDTHS = [16, 32] + [48] * 8 + [32, 16, 9]    # pair-blocks per super-block
NP2 = sum(WIDTHS)              # 489 pair-blocks
NJ = 2 * NP2                   # 978 j-blocks of 128 rows
R = 128 * NJ                   # per-core rows = 125184
B_PAD = R * NCORES             # 1001472
F32 = mybir.dt.float32
F16 = mybir.dt.float16
NPF16 = np.float16


# ---------------------------------------------------------------- host math
def _bs_pass(n, start, int_params):
    i = np.arange(start, n - 1, 2)
    j = i + 1
    theta = int_params[3 * i]
    phi = int_params[3 * i + 1]
    ct, st = np.cos(theta), np.sin(theta)
    cp, sp = np.cos(phi), np.sin(phi)
    S = np.eye(2 * n)
    S[i, i] = ct
    S[i, j] = -cp * st
    S[i, n + j] = -sp * st
    S[j, i] = cp * st
    S[j, j] = ct
    S[j, n + i] = -sp * st
    S[n + i, j] = sp * st
    S[n + i, n + i] = ct
    S[n + i, n + j] = -cp * st
    S[n + j, i] = sp * st
    S[n + j, n + i] = cp * st
    S[n + j, n + j] = ct
    return S


def _layer_symplectic(n, int1, squeezes, int2):
    M = _bs_pass(n, 0, int1)
    M = _bs_pass(n, 1, int1) @ M
    c = np.concatenate([np.cos(int1[2::3]), np.ones(1)])
    s = np.concatenate([np.sin(int1[2::3]), np.zeros(1)])
    Rm = np.block([[np.diag(c), np.diag(-s)], [np.diag(s), np.diag(c)]])
    Sq = np.diag(np.concatenate([np.exp(-squeezes), np.exp(squeezes)]))
    M = Sq @ (Rm @ M)
    M = _bs_pass(n, 0, int2) @ M
    M = _bs_pass(n, 1, int2) @ M
    return M


def _affine_map(layers):
    n = N
    S = np.eye(2 * n)
    d = np.zeros(2 * n)
    for int1, sq, int2, disp in layers:
        M = _layer_symplectic(n, int1, sq, int2)
        S = M @ S
        d = M @ d
        d[:n] += 2.0 * disp
    return S, d


def _device_constants(layers):
    S, d = _affine_map(layers)
    w = np.arange(OUT)
    rows = np.concatenate([w, N + w])
    cov = S @ S.T
    cov_term = cov[w, w] + cov[N + w, N + w]            # (10,)
    W2 = S[rows, :N].T                                  # (64, 20)
    d20 = d[rows] / 2.0                                 # (20,)
    # cross weights: CW[i,k] = 2(dx_k W2[i,k] + dp_k W2[i,10+k])
    CW = 2.0 * (W2[:, 0:OUT] * d20[0:OUT] + W2[:, OUT:] * d20[OUT:])
    # c' = cov/4 - 1/2 + dx^2 + dp^2
    cmod = cov_term / 4.0 - 0.5 + d20[0:OUT] ** 2 + d20[OUT:] ** 2

    w60 = np.zeros((128, 60), NPF16)
    w60[0:64, 0:20] = W2.astype(NPF16)
    w60[64:128, 20:40] = W2.astype(NPF16)
    w60[0:64, 40:50] = CW.astype(NPF16)
    w60[64:128, 50:60] = CW.astype(NPF16)

    cconst = np.ascontiguousarray(np.broadcast_to(
        np.tile(cmod.astype(NPF16), 2 * SBP), (128, 20 * SBP))).astype(NPF16)
    return w60, cconst


# ---------------------------------------------------------------- bass build
def build_nc(widths=None):
    widths = widths or WIDTHS
    np2 = sum(widths)
    cc = 128 * np2                              # xstack cols
    nc = bacc.Bacc("TRN2", target_bir_lowering=False)
    OC = 20 * SBP                               # out cols per full SB (640)
    xs = nc.dram_tensor("xs", (128, cc), F16, kind="ExternalInput")
    wst = nc.dram_tensor("w60", (128, 60), F16, kind="ExternalInput")
    ccon = nc.dram_tensor("covconst", (128, OC), F16, kind="ExternalInput")
    out = nc.dram_tensor("out", (128, 2 * np2 * OUT), F16,
                         kind="ExternalOutput")

    Square = mybir.ActivationFunctionType.Square
    Ln = mybir.ActivationFunctionType.Ln

    with tile.TileContext(nc) as tc:
        with (
            tc.tile_pool(name="const", bufs=1) as cpool,
            tc.tile_pool(name="xin", bufs=8) as xpool,
            tc.tile_pool(name="mid", bufs=4) as mpool,
            tc.tile_pool(name="ob", bufs=4) as opool,
            tc.tile_pool(name="ps", bufs=4, space="PSUM") as pspool,
        ):
            # consts on the scalar queue: the sync queue is a pure x stream
            w_t = cpool.tile([128, 60], F16)
            nc.scalar.dma_start(w_t[:], wst[:])
            c_t = cpool.tile([128, OC], F16)
            nc.scalar.dma_start(c_t[:], ccon[:])

            def emit_tail(prev):
                # tail of the PREVIOUS super-block: its u is long done, so
                # none of these ops stalls at the head of an in-order
                # engine queue (a stalled op blocks everything emitted
                # after it on the same queue).
                u, col_base, jblk = prev
                oc = 20 * jblk
                v = mpool.tile([128, oc], F16, tag="v")
                nc.vector.tensor_add(v[:], u[:], c_t[:, 0:oc])
                o = opool.tile([128, oc], F16, tag="o")
                nc.scalar.activation(o[:], v[:], Ln, bias=1.0)
                ob = (col_base // 128) * 20
                nc.gpsimd.dma_start(out[:, ob:ob + oc], o[:])

            def emit_chunk(tin, b0, cp, sq, u):
                # one psum chunk (<= 16 pair-blocks, 2 banks): matmuls,
                # then Square (ACT) and cross-add (DVE) drain it.  psum:
                # 8 pair-blocks of 60 cols in the first 480 cols of each
                # 512-col bank (no bank crossing).
                nbank = (cp + PPB - 1) // PPB
                ps = pspool.tile([128, nbank, 512], F32, tag="ps")
                for j in range(cp):
                    nc.tensor.matmul(
                        ps[:, j // PPB, 60 * (j % PPB):60 * (j % PPB) + 60],
                        tin[:, 128 * (b0 + j):128 * (b0 + j) + 128], w_t[:],
                        start=True, stop=True,
                    )
                fullb = cp // PPB          # banks fully used
                remg = cp - fullb * PPB    # pair-blocks in the ragged bank
                so, uo = 40 * b0, 20 * b0
                if fullb:
                    pv = ps[:, 0:fullb, 0:60 * PPB].rearrange(
                        "p t (g c) -> p t g c", c=60)[:, :, :, 0:40]
                    sv = sq[:, so:so + 40 * PPB * fullb].rearrange(
                        "p (t g c) -> p t g c", g=PPB, c=40)
                    nc.scalar.activation(sv, pv, Square)
                if remg:
                    pv = ps[:, fullb, 0:60 * remg].rearrange(
                        "p (g c) -> p g c", c=60)[:, :, 0:40]
                    sv = sq[:, so + 40 * PPB * fullb:so + 40 * cp].rearrange(
                        "p (g c) -> p g c", c=40)
                    nc.scalar.activation(sv, pv, Square)

                sqv = sq[:, so:so + 40 * cp].rearrange(
                    "p (g r k) -> p g r k", r=2, k=OUT)
                s = mpool.tile([128, 20 * cp], F16, tag="s")
                sv = s[:].rearrange("p (g k) -> p g k", k=OUT)
                nc.vector.tensor_add(sv, sqv[:, :, 0, :], sqv[:, :, 1, :])
                if fullb:
                    pc = ps[:, 0:fullb, 0:60 * PPB].rearrange(
                        "p t (g c) -> p t g c", c=60)[:, :, :, 40:60]
                    uv = u[:, uo:uo + 20 * PPB * fullb].rearrange(
                        "p (t g c) -> p t g c", g=PPB, c=20)
                    sv2 = s[:, 0:20 * PPB * fullb].rearrange(
                        "p (t g c) -> p t g c", g=PPB, c=20)
                    nc.vector.tensor_add(uv, sv2, pc)
                if remg:
                    pc = ps[:, fullb, 0:60 * remg].rearrange(
                        "p (g c) -> p g c", c=60)[:, :, 40:60]
                    uv = u[:, uo + 20 * PPB * fullb:uo + 20 * cp].rearrange(
                        "p (g c) -> p g c", c=20)
                    sv2 = s[:, 20 * PPB * fullb:20 * cp].rearrange(
                        "p (g c) -> p g c", c=20)
                    nc.vector.tensor_add(uv, sv2, pc)

            def emit_sb(col_base, jblk, prev):
                w = 128 * jblk
                tin = xpool.tile([128, w], F16, tag="tin")
                u = mpool.tile([128, 20 * jblk], F16, tag="u")
                sq = mpool.tile([128, 40 * jblk], F16, tag="sq")
                # input DMA split per psum chunk so each chunk's matmuls
                # gate on just its slice of the transfer
                base = 0
                left = jblk
                first = True
                while left:
                    cp = min(CHK, left)
                    nc.sync.dma_start(
                        tin[:, 128 * base:128 * (base + cp)],
                        xs[:, col_base + 128 * base:
                           col_base + 128 * (base + cp)])
                    if first and prev is not None:
                        emit_tail(prev)
                        first = False
                    emit_chunk(tin, base, cp, sq, u)
                    base += cp
                    left -= cp
                if first and prev is not None:
                    emit_tail(prev)
                return (u, col_base, jblk)

            col = 0
            prev = None
            for wdt in widths:
                prev = emit_sb(col, wdt, prev)
                col += 128 * wdt
            emit_tail(prev)
    nc.compile()
    return nc


# ---------------------------------------------------------------- host glue
def _make_in_maps(x_batch, w60, cconst):
    B = x_batch.shape[0]
    xpad = np.zeros((B_PAD, N), NPF16)
    xpad[:B] = x_batch
    in_maps = []
    for c in range(NCORES):
        xc = xpad[c * R:(c + 1) * R]
        # xstk[64*m + f, 128*b + l] = xc[256*b + 128*m + l, f]
        xstk = np.ascontiguousarray(
            xc.reshape(R // 256, 2, 128, N).transpose(1, 3, 0, 2)
            .reshape(128, R // 2))
        in_maps.append({"xs": xstk, "w60": w60, "covconst": cconst})
    return in_maps


def _decode_out(results, B):
    full = np.empty((B_PAD, OUT), np.float32)
    for c in range(NCORES):
        O = results[c]["out"].astype(np.float32).reshape(128, NJ, OUT)
        rows = O.transpose(1, 0, 2).reshape(R, OUT)
        full[c * R:(c + 1) * R] = rows
    return full[:B]


_NC_CACHE = {}


def kernel(x_batch, int1_0, squeezes_0, int2_0, disp_0,
           int1_1, squeezes_1, int2_1, disp_1, _trace=False):
    layers = [
        (np.asarray(int1_0, np.float64), np.asarray(squeezes_0, np.float64),
         np.asarray(int2_0, np.float64), np.asarray(disp_0, np.float64)),
        (np.asarray(int1_1, np.float64), np.asarray(squeezes_1, np.float64),
         np.asarray(int2_1, np.float64), np.asarray(disp_1, np.float64)),
    ]
    w60, cconst = _device_constants(layers)
    in_maps = _make_in_maps(np.asarray(x_batch, np.float32), w60, cconst)

    if "nc" not in _NC_CACHE:
        _NC_CACHE["nc"] = build_nc()
    nc = _NC_CACHE["nc"]

    res = run_bass_kernel_spmd(
        nc, in_maps, core_ids=list(range(NCORES)), trace=_trace
    )
    out = _decode_out(res.results, x_batch.shape[0])
    if _trace:
        return out, res
    return out
